# revision 15
# baseline (speedup 1.0000x reference)
"""AdaptiveLIFNeuron Trainium2 kernel (8 NeuronCores, data-parallel over batch).

Reference recurrence per element (b, c, n), t = 0..T-1:
    m  = m * a + x_t          (a = exp(-1/tau_mem))
    s  = [m >= th]            (heaviside)
    m  = m - th * s           (reset by subtraction)
    th = th * b + s           (b = exp(-1/tau_thresh))
    out_t = s

Device strategy (per core, one batch element, E = C*N = 262144 elements laid
out as [128 partitions x 2048 free]):
  - GPSIMD (POOL) computes MP = G + X_t with a stock tensor_tensor add,
    running concurrently with the DVE (verified: DVE custom ops whose second
    operand is in PSUM leave the shared second SBUF port free).
  - DVE runs two runtime-registered custom ops per step:
      LIF_THR2: TH' = b*TH + (MP >= TH)       (TH state lives in PSUM)
      LIF_MEM:  G   = a*select(MP>=TH, MP-TH, MP)
  - ScalarE casts TH' (PSUM f32) -> bf16 SBUF, then its HWDGE ring DMAs the
    bf16 threshold sequence out. Spikes are reconstructed exactly on the
    host: s_t = [th_t - b*th_{t-1} > 0.5] (spike contribution is 1.0,
    bf16 rounding noise << 0.5).
  - Half-tile pipelining overlaps POOL(h1) with DVE(h0); 3-deep X prefetch
    absorbs in-DMA jitter.

x is pre-transposed on the host to [T, E] per core so each time step is one
contiguous 1 MiB DMA row. The kernel is bit-exact vs the f32 reference
(0 mismatched spikes / 67M on the harness input).
"""

import sys

if "/opt/trn_rl_repo" not in sys.path:
    sys.path.insert(0, "/opt/trn_rl_repo")

import numpy as np

B, C, N, T = 8, 64, 4096, 32
E = C * N  # 262144 elements per core
P = 128
F = E // P  # 2048
H = F // 2
NCORES = 8

_REG = {}
_COMPILED = {}


def _register_ops():
    """Register the fused LIF custom-DVE ops at runtime (append-only)."""
    if _REG:
        return _REG
    from concourse import dve_ops
    from concourse.dve_spec import C0, Spec, Src0, Src1, lower, select
    from concourse.dve_uop import DveOpSpec

    existing = {op.name: op for op in dve_ops.OPS}

    def reg(name, spec):
        if name in existing:
            return existing[name]
        row = dve_ops._CUSTOM_DVE_ROW_BASE + len(dve_ops.OPS)
        assert row < 0x20, "custom DVE row field overflow"
        dve_ops._SUB_OPCODE_FOR_NAME[name] = row
        shas = {}
        for ver in ("v3", "v4"):
            s = DveOpSpec(
                name=name,
                opcode=row,
                uops=lower(spec, ver=ver),
                rd1_en=dve_ops.has_src1(spec),
            )
            shas[ver] = s.sha(ver)
        op = dve_ops.DveOp(name, spec, False, uops_sha=shas)
        dve_ops.OPS.append(op)
        dve_ops.CUSTOM_DVE_SPECS[name] = spec
        return op

    # out = a * (in0 - in1 if in0 >= in1 else in0)   (in0=MP, in1=TH, s0=a)
    lif_mem = reg(
        "LIF_MEM_ANT",
        Spec(
            body=select(Src0 >= Src1, Src0 - Src1, Src0) * C0,
            reference=lambda in0, in1, s0, s1, imm2: (
                np.where(in0 >= in1, in0 - in1, in0) * np.float32(s0)
            ).astype(np.float32),
        ),
    )
    # out = b * in0 + (in1 >= in0)                   (in0=TH, in1=MP, s0=b)
    lif_thr = reg(
        "LIF_THR_ANT",
        Spec(
            body=Src0 * C0 + (Src1 >= Src0),
            reference=lambda in0, in1, s0, s1, imm2: (
                in0 * np.float32(s0) + (in1 >= in0).astype(np.float32)
            ).astype(np.float32),
        ),
    )
    # out = b * in1 + (in0 >= in1)                   (in0=MP, in1=TH, s0=b)
    # operand-swapped variant of LIF_THR so TH can ride the PSUM port
    lif_thr2 = reg(
        "LIF_THR2_ANT",
        Spec(
            body=Src1 * C0 + (Src0 >= Src1),
            reference=lambda in0, in1, s0, s1, imm2: (
                in1 * np.float32(s0) + (in0 >= in1).astype(np.float32)
            ).astype(np.float32),
        ),
    )
    _REG["mem"] = lif_mem
    _REG["thr"] = lif_thr
    _REG["thr2"] = lif_thr2
    return _REG


def _build(a, b):
    """Build the SPMD Bass graph. a/b are python floats holding exact f32
    decay values; they are baked into the custom-op scalar immediates."""
    import concourse.bass as bass
    import concourse.mybir as mybir

    ops = _register_ops()

    nc = bass.Bass(trn_type="TRN2")
    x = nc.declare_dram_parameter("x", [T, P, F], mybir.dt.float32, isOutput=False)
    out = nc.declare_dram_parameter("out", [T, P, F], mybir.dt.bfloat16, isOutput=True)

    from contextlib import ExitStack

    es = ExitStack()
    sb = lambda n, dt: es.enter_context(nc.sbuf_tensor(n, [P, F], dt))
    xb = [
        sb("x0", mybir.dt.float32),
        sb("x1", mybir.dt.float32),
        sb("x2", mybir.dt.float32),
    ]
    g = sb("g", mybir.dt.float32)
    mp = sb("mp", mybir.dt.float32)
    th = [
        es.enter_context(nc.psum_tensor("th0", [P, F], mybir.dt.float32)),
        es.enter_context(nc.psum_tensor("th1", [P, F], mybir.dt.float32)),
    ]
    tho = [sb("tho0", mybir.dt.bfloat16), sb("tho1", mybir.dt.bfloat16)]
    din = es.enter_context(nc.semaphore("din"))
    dinb = es.enter_context(nc.semaphore("dinb"))
    dout = es.enter_context(nc.semaphore("dout"))
    vsem = es.enter_context(nc.semaphore("vsem"))
    asem = es.enter_context(nc.semaphore("asem"))
    psem = es.enter_context(nc.semaphore("psem"))

    def half(buf, h):
        return buf[:, h * H : (h + 1) * H]

    with es, nc.Block() as block:

        @block.sync
        def _(sync):
            # t=0 load is split across the two HWDGE rings (h0 here, h1 on
            # the scalar ring) so the first DVE pair starts sooner
            sync.dma_start(half(xb[0], 0), x[0, :, 0:H]).then_inc(din, 16)
            for t in range(1, T):
                if t == 3:
                    # xb[0] was read by the t=0 DVE pairs (not POOL)
                    sync.wait_ge(vsem, 2)
                elif t >= 4:
                    # X buffer t%3 last read by POOL adds of step t-3
                    sync.wait_ge(psem, 2 * (t - 4) + 2)
                sync.dma_start(xb[t % 3][:, :], x[t, :, :]).then_inc(din, 16)

        @block.gpsimd
        def _(gpsimd):
            # step 0 has no ADD: the DVE pairs read X_0 directly (G = 0)
            for t in range(1, T):
                for h in range(2):
                    # G[h]/MP[h] free after the (h,t-1) DVE pair
                    gpsimd.wait_ge(vsem, 2 * (t - 1) + h + 1)
                    if h == 0:
                        gpsimd.wait_ge(din, 16 * (t + 1))
                    gpsimd.tensor_tensor(
                        half(mp, h),
                        half(g, h),
                        half(xb[t % 3], h),
                        mybir.AluOpType.add,
                    ).then_inc(psem, 1)

        @block.vector
        def _(vector):
            # split theta memset so the h0 pair isn't gated on the h1 half;
            # G needs no init (first write is MEM at t=0)
            nc.vector.memset(half(th[0], 0), 1.0)
            nc.vector.memset(half(th[0], 1), 1.0)
            for t in range(T):
                for h in range(2):
                    if t == 0:
                        # read X_0 directly: MP would be 0 + X_0
                        vector.wait_ge(din if h == 0 else dinb, 16)
                        mp_src = half(xb[0], h)
                    else:
                        vector.wait_ge(psem, 2 * (t - 1) + h + 1)
                        mp_src = half(mp, h)
                    if t >= 2:
                        # Op3 writes TH[(t+1)%2][h]; ACT copy(h,t-2) read it
                        vector.wait_ge(asem, 2 * (t - 2) + h + 1)
                    # Op3: TH' = b*TH + (MP >= TH)
                    thr_inst = nc.vector._custom_dve(
                        ops["thr2"],
                        out=half(th[(t + 1) % 2], h),
                        in0=mp_src,
                        in1=half(th[t % 2], h),
                        s0=b,
                    )
                    if t == T - 1:
                        # G is dead after the last step: skip MEM
                        thr_inst.then_inc(vsem, 1)
                    else:
                        # Op2: G = a * select(MP >= TH, MP - TH, MP)
                        nc.vector._custom_dve(
                            ops["mem"],
                            out=half(g, h),
                            in0=mp_src,
                            in1=half(th[t % 2], h),
                            s0=a,
                        ).then_inc(vsem, 1)

        @block.scalar
        def _(scalar):
            # second half of the t=0 input load rides this ring
            scalar.dma_start(half(xb[0], 1), x[0, :, H:F]).then_inc(dinb, 16)
            acnt = 0
            for t in range(T):
                for h in range(2):
                    scalar.wait_ge(vsem, 2 * t + h + 1)
                    if t >= 2:
                        scalar.wait_ge(dout, 16 * (2 * (t - 2) + h + 1))
                    # downcast f32 -> bf16 on ScalarE, then DMA out. The HWDGE
                    # descriptor post is a sequencer op that runs ahead of the
                    # ACT datapath, so self-wait on the copy's completion
                    # event before issuing the DMA.
                    if t == T - 1:
                        # final step: quarter-granularity so the last DMA
                        # transfer overlaps the remaining casts (shorter tail)
                        Q = H // 2
                        for q in range(2):
                            lo = h * H + q * Q
                            sl = slice(lo, lo + Q)
                            nc.scalar.copy(
                                tho[t % 2][:, sl], th[(t + 1) % 2][:, sl]
                            ).then_inc(asem, 1)
                            acnt += 1
                            scalar.wait_ge(asem, acnt)
                            scalar.dma_start(
                                out[t, :, sl], tho[t % 2][:, sl]
                            ).then_inc(dout, 16)
                    else:
                        nc.scalar.copy(
                            half(tho[t % 2], h), half(th[(t + 1) % 2], h)
                        ).then_inc(asem, 1)
                        acnt += 1
                        scalar.wait_ge(asem, acnt)
                        scalar.dma_start(
                            out[t, :, h * H : (h + 1) * H], half(tho[t % 2], h)
                        ).then_inc(dout, 16)

    # populate .instr bytes for InstISA subclasses (InstCustomDveAnt) —
    # raw Bass skips this pass; without it walrus fails "ISA wrong length"
    mybir.codegen_inst_isa_subclasses(nc)
    return nc


def _get_compiled(a, b):
    key = (a, b)
    if key not in _COMPILED:
        _COMPILED[key] = _build(a, b)
    return _COMPILED[key]


def _decays(tau_mem, tau_thresh):
    # match the reference bit-for-bit: f32 exp(-1/tau) computed via jax
    import jax.numpy as jnp

    a = float(jnp.exp(-1.0 / jnp.asarray(tau_mem, jnp.float32)))
    b = float(jnp.exp(-1.0 / jnp.asarray(tau_thresh, jnp.float32)))
    return a, b


def kernel(x, tau_mem, tau_thresh):
    from concourse.bass_utils import run_bass_kernel_spmd

    a, b = _decays(tau_mem, tau_thresh)
    x = np.asarray(x, dtype=np.float32)
    assert x.shape == (B, C, N, T), x.shape

    in_maps = []
    for i in range(NCORES):
        # [C,N,T] -> [E,T] -> [T,E] contiguous -> [T,P,F]
        xi = np.ascontiguousarray(x[i].reshape(E, T).T).reshape(T, P, F)
        in_maps.append({"x": xi})

    res = run_bass_kernel_spmd(_get_compiled(a, b), in_maps, list(range(NCORES)))

    out = np.empty((B, C, N, T), dtype=np.float32)
    for i in range(NCORES):
        th = np.asarray(res.results[i]["out"]).astype(np.float64).reshape(T, E)
        thprev = np.empty_like(th)
        thprev[0] = 1.0
        thprev[1:] = th[:-1]
        s = (th - b * thprev) > 0.5  # [T, E]
        out[i] = s.T.astype(np.float32).reshape(C, N, T)
    return out


# revision 16
# speedup vs baseline: 1.1860x; 1.1860x over previous
"""AdaptiveLIFNeuron Trainium2 kernel (8 NeuronCores, data-parallel over batch).

Reference recurrence per element (b, c, n), t = 0..T-1:
    m  = m * a + x_t          (a = exp(-1/tau_mem))
    s  = [m >= th]            (heaviside)
    m  = m - th * s           (reset by subtraction)
    th = th * b + s           (b = exp(-1/tau_thresh))
    out_t = s

Device strategy (per core, one batch element, E = C*N = 262144 elements laid
out as [128 partitions x 2048 free]):
  - GPSIMD (POOL) computes MP = G + X_t with a stock tensor_tensor add,
    running concurrently with the DVE (verified: DVE custom ops whose second
    operand is in PSUM leave the shared second SBUF port free).
  - DVE runs two runtime-registered custom ops per step:
      LIF_THR2: TH' = b*TH + (MP >= TH)       (TH state lives in PSUM)
      LIF_MEM:  G   = a*select(MP>=TH, MP-TH, MP)
  - ScalarE casts TH' (PSUM f32) -> bf16 SBUF, then its HWDGE ring DMAs the
    bf16 threshold sequence out. Spikes are reconstructed exactly on the
    host: s_t = [th_t - b*th_{t-1} > 0.5] (spike contribution is 1.0,
    bf16 rounding noise << 0.5).
  - Half-tile pipelining overlaps POOL(h1) with DVE(h0); 3-deep X prefetch
    absorbs in-DMA jitter.

x is pre-transposed on the host to [T, E] per core so each time step is one
contiguous 1 MiB DMA row. The kernel is bit-exact vs the f32 reference
(0 mismatched spikes / 67M on the harness input).
"""

import sys

if "/opt/trn_rl_repo" not in sys.path:
    sys.path.insert(0, "/opt/trn_rl_repo")

import numpy as np

B, C, N, T = 8, 64, 4096, 32
E = C * N  # 262144 elements per core
P = 128
F = E // P  # 2048
H = F // 2
NCORES = 8

_REG = {}
_COMPILED = {}


def _register_ops():
    """Register the fused LIF custom-DVE ops at runtime (append-only)."""
    if _REG:
        return _REG
    from concourse import dve_ops
    from concourse.dve_spec import C0, Spec, Src0, Src1, lower, select
    from concourse.dve_uop import DveOpSpec

    existing = {op.name: op for op in dve_ops.OPS}

    def reg(name, spec):
        if name in existing:
            return existing[name]
        row = dve_ops._CUSTOM_DVE_ROW_BASE + len(dve_ops.OPS)
        assert row < 0x20, "custom DVE row field overflow"
        dve_ops._SUB_OPCODE_FOR_NAME[name] = row
        shas = {}
        for ver in ("v3", "v4"):
            s = DveOpSpec(
                name=name,
                opcode=row,
                uops=lower(spec, ver=ver),
                rd1_en=dve_ops.has_src1(spec),
            )
            shas[ver] = s.sha(ver)
        op = dve_ops.DveOp(name, spec, False, uops_sha=shas)
        dve_ops.OPS.append(op)
        dve_ops.CUSTOM_DVE_SPECS[name] = spec
        return op

    # out = a * (in0 - in1 if in0 >= in1 else in0)   (in0=MP, in1=TH, s0=a)
    lif_mem = reg(
        "LIF_MEM_ANT",
        Spec(
            body=select(Src0 >= Src1, Src0 - Src1, Src0) * C0,
            reference=lambda in0, in1, s0, s1, imm2: (
                np.where(in0 >= in1, in0 - in1, in0) * np.float32(s0)
            ).astype(np.float32),
        ),
    )
    # out = b * in0 + (in1 >= in0)                   (in0=TH, in1=MP, s0=b)
    lif_thr = reg(
        "LIF_THR_ANT",
        Spec(
            body=Src0 * C0 + (Src1 >= Src0),
            reference=lambda in0, in1, s0, s1, imm2: (
                in0 * np.float32(s0) + (in1 >= in0).astype(np.float32)
            ).astype(np.float32),
        ),
    )
    # out = b * in1 + (in0 >= in1)                   (in0=MP, in1=TH, s0=b)
    # operand-swapped variant of LIF_THR so TH can ride the PSUM port
    lif_thr2 = reg(
        "LIF_THR2_ANT",
        Spec(
            body=Src1 * C0 + (Src0 >= Src1),
            reference=lambda in0, in1, s0, s1, imm2: (
                in1 * np.float32(s0) + (in0 >= in1).astype(np.float32)
            ).astype(np.float32),
        ),
    )
    _REG["mem"] = lif_mem
    _REG["thr"] = lif_thr
    _REG["thr2"] = lif_thr2
    return _REG


def _build(a, b):
    """Build the SPMD Bass graph. a/b are python floats holding exact f32
    decay values; they are baked into the custom-op scalar immediates."""
    import concourse.bass as bass
    import concourse.mybir as mybir

    ops = _register_ops()

    nc = bass.Bass(trn_type="TRN2")
    x = nc.declare_dram_parameter("x", [T, P, F], mybir.dt.float32, isOutput=False)
    out = nc.declare_dram_parameter("out", [T, P, F], mybir.dt.bfloat16, isOutput=True)

    from contextlib import ExitStack

    es = ExitStack()
    sb = lambda n, dt: es.enter_context(nc.sbuf_tensor(n, [P, F], dt))
    xb = [
        sb("x0", mybir.dt.float32),
        sb("x1", mybir.dt.float32),
        sb("x2", mybir.dt.float32),
    ]
    g = sb("g", mybir.dt.float32)
    mp = sb("mp", mybir.dt.float32)
    th = [
        es.enter_context(nc.psum_tensor("th0", [P, F], mybir.dt.float32)),
        es.enter_context(nc.psum_tensor("th1", [P, F], mybir.dt.float32)),
    ]
    tho = [sb("tho0", mybir.dt.bfloat16), sb("tho1", mybir.dt.bfloat16)]
    din = es.enter_context(nc.semaphore("din"))
    dinb = es.enter_context(nc.semaphore("dinb"))
    dout = es.enter_context(nc.semaphore("dout"))
    vsem = es.enter_context(nc.semaphore("vsem"))
    asem = es.enter_context(nc.semaphore("asem"))
    psem = es.enter_context(nc.semaphore("psem"))

    def half(buf, h):
        return buf[:, h * H : (h + 1) * H]

    VI = 2  # vsem init offset (two memsets)

    with es, nc.Block() as block:

        @block.sync
        def _(sync):
            # t=0 load is split across the two HWDGE rings (h0 here, h1 on
            # the scalar ring) so the first POOL add starts sooner
            sync.dma_start(half(xb[0], 0), x[0, :, 0:H]).then_inc(din, 16)
            for t in range(1, T):
                if t >= 3:
                    # X buffer t%3 last read by POOL adds of step t-3
                    sync.wait_ge(psem, 2 * (t - 3) + 2)
                sync.dma_start(xb[t % 3][:, :], x[t, :, :]).then_inc(din, 16)

        @block.gpsimd
        def _(gpsimd):
            for t in range(T):
                for h in range(2):
                    if t == 0:
                        gpsimd.wait_ge(vsem, VI)  # memsets done
                        if h == 0:
                            gpsimd.wait_ge(din, 16)
                        else:
                            gpsimd.wait_ge(dinb, 16)
                    else:
                        # G[h]/MP[h] free after the (h,t-1) DVE pair
                        gpsimd.wait_ge(vsem, VI + 2 * (t - 1) + h + 1)
                        if h == 0:
                            gpsimd.wait_ge(din, 16 * (t + 1))
                    gpsimd.tensor_tensor(
                        half(mp, h),
                        half(g, h),
                        half(xb[t % 3], h),
                        mybir.AluOpType.add,
                    ).then_inc(psem, 1)

        @block.vector
        def _(vector):
            nc.vector.memset(g[:, :], 0.0).then_inc(vsem, 1)
            nc.vector.memset(th[0][:, :], 1.0).then_inc(vsem, 1)
            for t in range(T):
                for h in range(2):
                    vector.wait_ge(psem, 2 * t + h + 1)
                    if t >= 2:
                        # Op3 writes TH[(t+1)%2][h]; ACT copy(h,t-2) read it
                        vector.wait_ge(asem, 2 * (t - 2) + h + 1)
                    # Op3: TH' = b*TH + (MP >= TH)
                    thr_inst = nc.vector._custom_dve(
                        ops["thr2"],
                        out=half(th[(t + 1) % 2], h),
                        in0=half(mp, h),
                        in1=half(th[t % 2], h),
                        s0=b,
                    )
                    if t == T - 1:
                        # G is dead after the last step: skip MEM
                        thr_inst.then_inc(vsem, 1)
                    else:
                        # Op2: G = a * select(MP >= TH, MP - TH, MP)
                        nc.vector._custom_dve(
                            ops["mem"],
                            out=half(g, h),
                            in0=half(mp, h),
                            in1=half(th[t % 2], h),
                            s0=a,
                        ).then_inc(vsem, 1)

        @block.scalar
        def _(scalar):
            # second half of the t=0 input load rides this ring
            scalar.dma_start(half(xb[0], 1), x[0, :, H:F]).then_inc(dinb, 16)
            for t in range(T):
                for h in range(2):
                    scalar.wait_ge(vsem, VI + 2 * t + h + 1)
                    if t >= 2:
                        scalar.wait_ge(dout, 16 * (2 * (t - 2) + h + 1))
                    # downcast f32 -> bf16 on ScalarE, then DMA out. The HWDGE
                    # descriptor post is a sequencer op that runs ahead of the
                    # ACT datapath, so self-wait on the copy's completion
                    # event before issuing the DMA.
                    nc.scalar.copy(
                        half(tho[t % 2], h), half(th[(t + 1) % 2], h)
                    ).then_inc(asem, 1)
                    scalar.wait_ge(asem, 2 * t + h + 1)
                    scalar.dma_start(
                        out[t, :, h * H : (h + 1) * H], half(tho[t % 2], h)
                    ).then_inc(dout, 16)

    # populate .instr bytes for InstISA subclasses (InstCustomDveAnt) —
    # raw Bass skips this pass; without it walrus fails "ISA wrong length"
    mybir.codegen_inst_isa_subclasses(nc)
    return nc


def _get_compiled(a, b):
    key = (a, b)
    if key not in _COMPILED:
        _COMPILED[key] = _build(a, b)
    return _COMPILED[key]


def _decays(tau_mem, tau_thresh):
    # match the reference bit-for-bit: f32 exp(-1/tau) computed via jax
    import jax.numpy as jnp

    a = float(jnp.exp(-1.0 / jnp.asarray(tau_mem, jnp.float32)))
    b = float(jnp.exp(-1.0 / jnp.asarray(tau_thresh, jnp.float32)))
    return a, b


def kernel(x, tau_mem, tau_thresh):
    from concourse.bass_utils import run_bass_kernel_spmd

    a, b = _decays(tau_mem, tau_thresh)
    x = np.asarray(x, dtype=np.float32)
    assert x.shape == (B, C, N, T), x.shape

    in_maps = []
    for i in range(NCORES):
        # [C,N,T] -> [E,T] -> [T,E] contiguous -> [T,P,F]
        xi = np.ascontiguousarray(x[i].reshape(E, T).T).reshape(T, P, F)
        in_maps.append({"x": xi})

    res = run_bass_kernel_spmd(_get_compiled(a, b), in_maps, list(range(NCORES)))

    out = np.empty((B, C, N, T), dtype=np.float32)
    for i in range(NCORES):
        th = np.asarray(res.results[i]["out"]).astype(np.float64).reshape(T, E)
        thprev = np.empty_like(th)
        thprev[0] = 1.0
        thprev[1:] = th[:-1]
        s = (th - b * thprev) > 0.5  # [T, E]
        out[i] = s.T.astype(np.float32).reshape(C, N, T)
    return out


# revision 17
# speedup vs baseline: 1.2004x; 1.0121x over previous
"""AdaptiveLIFNeuron Trainium2 kernel (8 NeuronCores, data-parallel over batch).

Reference recurrence per element (b, c, n), t = 0..T-1:
    m  = m * a + x_t          (a = exp(-1/tau_mem))
    s  = [m >= th]            (heaviside)
    m  = m - th * s           (reset by subtraction)
    th = th * b + s           (b = exp(-1/tau_thresh))
    out_t = s

Device strategy (per core, one batch element, E = C*N = 262144 elements laid
out as [128 partitions x 2048 free]):
  - GPSIMD (POOL) computes MP = G + X_t with a stock tensor_tensor add,
    running concurrently with the DVE (verified: DVE custom ops whose second
    operand is in PSUM leave the shared second SBUF port free).
  - DVE runs two runtime-registered custom ops per step:
      LIF_THR2: TH' = b*TH + (MP >= TH)       (TH state lives in PSUM)
      LIF_MEM:  G   = a*select(MP>=TH, MP-TH, MP)
  - ScalarE casts TH' (PSUM f32) -> bf16 SBUF, then its HWDGE ring DMAs the
    bf16 threshold sequence out. Spikes are reconstructed exactly on the
    host: s_t = [th_t - b*th_{t-1} > 0.5] (spike contribution is 1.0,
    bf16 rounding noise << 0.5).
  - Half-tile pipelining overlaps POOL(h1) with DVE(h0); 3-deep X prefetch
    absorbs in-DMA jitter.

x is pre-transposed on the host to [T, E] per core so each time step is one
contiguous 1 MiB DMA row. The kernel is bit-exact vs the f32 reference
(0 mismatched spikes / 67M on the harness input).
"""

import sys

if "/opt/trn_rl_repo" not in sys.path:
    sys.path.insert(0, "/opt/trn_rl_repo")

import numpy as np

B, C, N, T = 8, 64, 4096, 32
E = C * N  # 262144 elements per core
P = 128
F = E // P  # 2048
H = F // 2
NCORES = 8

_REG = {}
_COMPILED = {}


def _register_ops():
    """Register the fused LIF custom-DVE ops at runtime (append-only)."""
    if _REG:
        return _REG
    from concourse import dve_ops
    from concourse.dve_spec import C0, Spec, Src0, Src1, lower, select
    from concourse.dve_uop import DveOpSpec

    existing = {op.name: op for op in dve_ops.OPS}

    def reg(name, spec):
        if name in existing:
            return existing[name]
        row = dve_ops._CUSTOM_DVE_ROW_BASE + len(dve_ops.OPS)
        assert row < 0x20, "custom DVE row field overflow"
        dve_ops._SUB_OPCODE_FOR_NAME[name] = row
        shas = {}
        for ver in ("v3", "v4"):
            s = DveOpSpec(
                name=name,
                opcode=row,
                uops=lower(spec, ver=ver),
                rd1_en=dve_ops.has_src1(spec),
            )
            shas[ver] = s.sha(ver)
        op = dve_ops.DveOp(name, spec, False, uops_sha=shas)
        dve_ops.OPS.append(op)
        dve_ops.CUSTOM_DVE_SPECS[name] = spec
        return op

    # out = a * (in0 - in1 if in0 >= in1 else in0)   (in0=MP, in1=TH, s0=a)
    lif_mem = reg(
        "LIF_MEM_ANT",
        Spec(
            body=select(Src0 >= Src1, Src0 - Src1, Src0) * C0,
            reference=lambda in0, in1, s0, s1, imm2: (
                np.where(in0 >= in1, in0 - in1, in0) * np.float32(s0)
            ).astype(np.float32),
        ),
    )
    # out = b * in0 + (in1 >= in0)                   (in0=TH, in1=MP, s0=b)
    lif_thr = reg(
        "LIF_THR_ANT",
        Spec(
            body=Src0 * C0 + (Src1 >= Src0),
            reference=lambda in0, in1, s0, s1, imm2: (
                in0 * np.float32(s0) + (in1 >= in0).astype(np.float32)
            ).astype(np.float32),
        ),
    )
    # out = b * in1 + (in0 >= in1)                   (in0=MP, in1=TH, s0=b)
    # operand-swapped variant of LIF_THR so TH can ride the PSUM port
    lif_thr2 = reg(
        "LIF_THR2_ANT",
        Spec(
            body=Src1 * C0 + (Src0 >= Src1),
            reference=lambda in0, in1, s0, s1, imm2: (
                in1 * np.float32(s0) + (in0 >= in1).astype(np.float32)
            ).astype(np.float32),
        ),
    )
    _REG["mem"] = lif_mem
    _REG["thr"] = lif_thr
    _REG["thr2"] = lif_thr2
    return _REG


def _build(a, b):
    """Build the SPMD Bass graph. a/b are python floats holding exact f32
    decay values; they are baked into the custom-op scalar immediates."""
    import concourse.bass as bass
    import concourse.mybir as mybir

    ops = _register_ops()

    nc = bass.Bass(trn_type="TRN2")
    x = nc.declare_dram_parameter("x", [T, P, F], mybir.dt.float32, isOutput=False)
    out = nc.declare_dram_parameter("out", [T, P, F], mybir.dt.bfloat16, isOutput=True)

    from contextlib import ExitStack

    es = ExitStack()
    sb = lambda n, dt: es.enter_context(nc.sbuf_tensor(n, [P, F], dt))
    xb = [
        sb("x0", mybir.dt.float32),
        sb("x1", mybir.dt.float32),
        sb("x2", mybir.dt.float32),
    ]
    g = sb("g", mybir.dt.float32)
    mp = sb("mp", mybir.dt.float32)
    th = [
        es.enter_context(nc.psum_tensor("th0", [P, F], mybir.dt.float32)),
        es.enter_context(nc.psum_tensor("th1", [P, F], mybir.dt.float32)),
    ]
    tho = [sb("tho0", mybir.dt.bfloat16), sb("tho1", mybir.dt.bfloat16)]
    din = es.enter_context(nc.semaphore("din"))
    dinb = es.enter_context(nc.semaphore("dinb"))
    dout = es.enter_context(nc.semaphore("dout"))
    vsem = es.enter_context(nc.semaphore("vsem"))
    asem = es.enter_context(nc.semaphore("asem"))
    psem = es.enter_context(nc.semaphore("psem"))

    def half(buf, h):
        return buf[:, h * H : (h + 1) * H]

    VI = 2  # vsem init offset (two memsets)

    with es, nc.Block() as block:

        @block.sync
        def _(sync):
            # t=0 load is split across the two HWDGE rings (h0 here, h1 on
            # the scalar ring) so the first POOL add starts sooner
            sync.dma_start(half(xb[0], 0), x[0, :, 0:H]).then_inc(din, 16)
            for t in range(1, T):
                if t >= 3:
                    # X buffer t%3 last read by POOL adds of step t-3
                    sync.wait_ge(psem, 2 * (t - 3) + 2)
                sync.dma_start(xb[t % 3][:, :], x[t, :, :]).then_inc(din, 16)

        @block.gpsimd
        def _(gpsimd):
            for t in range(T):
                for h in range(2):
                    if t == 0:
                        gpsimd.wait_ge(vsem, VI)  # memsets done
                        if h == 0:
                            gpsimd.wait_ge(din, 16)
                        else:
                            gpsimd.wait_ge(dinb, 16)
                    else:
                        # G[h]/MP[h] free after the (h,t-1) DVE pair
                        gpsimd.wait_ge(vsem, VI + 2 * (t - 1) + h + 1)
                        if h == 0:
                            gpsimd.wait_ge(din, 16 * (t + 1))
                    gpsimd.tensor_tensor(
                        half(mp, h),
                        half(g, h),
                        half(xb[t % 3], h),
                        mybir.AluOpType.add,
                    ).then_inc(psem, 1)

        @block.vector
        def _(vector):
            nc.vector.memset(g[:, :], 0.0).then_inc(vsem, 1)
            nc.vector.memset(th[0][:, :], 1.0).then_inc(vsem, 1)
            for t in range(T):
                for h in range(2):
                    vector.wait_ge(psem, 2 * t + h + 1)
                    if t >= 2:
                        # Op3 writes TH[(t+1)%2][h]; ACT copy(h,t-2) read it
                        vector.wait_ge(asem, 2 * (t - 2) + h + 1)
                    # Op3: TH' = b*TH + (MP >= TH). At t = T-1 theta is
                    # pure output (no later step reads it), so write bf16
                    # straight to the staging buffer and skip the ACT cast.
                    thr_inst = nc.vector._custom_dve(
                        ops["thr2"],
                        out=half(tho[(T - 1) % 2], h)
                        if t == T - 1
                        else half(th[(t + 1) % 2], h),
                        in0=half(mp, h),
                        in1=half(th[t % 2], h),
                        s0=b,
                    )
                    if t == T - 1:
                        # G is dead after the last step: skip MEM
                        thr_inst.then_inc(vsem, 1)
                    else:
                        # Op2: G = a * select(MP >= TH, MP - TH, MP)
                        nc.vector._custom_dve(
                            ops["mem"],
                            out=half(g, h),
                            in0=half(mp, h),
                            in1=half(th[t % 2], h),
                            s0=a,
                        ).then_inc(vsem, 1)

        @block.scalar
        def _(scalar):
            # second half of the t=0 input load rides this ring
            scalar.dma_start(half(xb[0], 1), x[0, :, H:F]).then_inc(dinb, 16)
            for t in range(T):
                for h in range(2):
                    scalar.wait_ge(vsem, VI + 2 * t + h + 1)
                    if t >= 2:
                        scalar.wait_ge(dout, 16 * (2 * (t - 2) + h + 1))
                    if t == T - 1:
                        # THR2 wrote bf16 into tho directly; vsem-gated DMA
                        # of DVE output is the original (proven) pattern
                        scalar.dma_start(
                            out[t, :, h * H : (h + 1) * H], half(tho[t % 2], h)
                        ).then_inc(dout, 16)
                        continue
                    # downcast f32 -> bf16 on ScalarE, then DMA out. The HWDGE
                    # descriptor post is a sequencer op that runs ahead of the
                    # ACT datapath, so self-wait on the copy's completion
                    # event before issuing the DMA.
                    nc.scalar.copy(
                        half(tho[t % 2], h), half(th[(t + 1) % 2], h)
                    ).then_inc(asem, 1)
                    scalar.wait_ge(asem, 2 * t + h + 1)
                    scalar.dma_start(
                        out[t, :, h * H : (h + 1) * H], half(tho[t % 2], h)
                    ).then_inc(dout, 16)

    # populate .instr bytes for InstISA subclasses (InstCustomDveAnt) —
    # raw Bass skips this pass; without it walrus fails "ISA wrong length"
    mybir.codegen_inst_isa_subclasses(nc)
    return nc


def _get_compiled(a, b):
    key = (a, b)
    if key not in _COMPILED:
        _COMPILED[key] = _build(a, b)
    return _COMPILED[key]


def _decays(tau_mem, tau_thresh):
    # match the reference bit-for-bit: f32 exp(-1/tau) computed via jax
    import jax.numpy as jnp

    a = float(jnp.exp(-1.0 / jnp.asarray(tau_mem, jnp.float32)))
    b = float(jnp.exp(-1.0 / jnp.asarray(tau_thresh, jnp.float32)))
    return a, b


def kernel(x, tau_mem, tau_thresh):
    from concourse.bass_utils import run_bass_kernel_spmd

    a, b = _decays(tau_mem, tau_thresh)
    x = np.asarray(x, dtype=np.float32)
    assert x.shape == (B, C, N, T), x.shape

    in_maps = []
    for i in range(NCORES):
        # [C,N,T] -> [E,T] -> [T,E] contiguous -> [T,P,F]
        xi = np.ascontiguousarray(x[i].reshape(E, T).T).reshape(T, P, F)
        in_maps.append({"x": xi})

    res = run_bass_kernel_spmd(_get_compiled(a, b), in_maps, list(range(NCORES)))

    out = np.empty((B, C, N, T), dtype=np.float32)
    for i in range(NCORES):
        th = np.asarray(res.results[i]["out"]).astype(np.float64).reshape(T, E)
        thprev = np.empty_like(th)
        thprev[0] = 1.0
        thprev[1:] = th[:-1]
        s = (th - b * thprev) > 0.5  # [T, E]
        out[i] = s.T.astype(np.float32).reshape(C, N, T)
    return out


# revision 18
# speedup vs baseline: 1.2065x; 1.0050x over previous
"""AdaptiveLIFNeuron Trainium2 kernel (8 NeuronCores, data-parallel over batch).

Reference recurrence per element (b, c, n), t = 0..T-1:
    m  = m * a + x_t          (a = exp(-1/tau_mem))
    s  = [m >= th]            (heaviside)
    m  = m - th * s           (reset by subtraction)
    th = th * b + s           (b = exp(-1/tau_thresh))
    out_t = s

Device strategy (per core, one batch element, E = C*N = 262144 elements laid
out as [128 partitions x 2048 free]):
  - GPSIMD (POOL) computes MP = G + X_t with a stock tensor_tensor add,
    running concurrently with the DVE (verified: DVE custom ops whose second
    operand is in PSUM leave the shared second SBUF port free).
  - DVE runs two runtime-registered custom ops per step:
      LIF_THR2: TH' = b*TH + (MP >= TH)       (TH state lives in PSUM)
      LIF_MEM:  G   = a*select(MP>=TH, MP-TH, MP)
  - ScalarE casts TH' (PSUM f32) -> bf16 SBUF, then its HWDGE ring DMAs the
    bf16 threshold sequence out. Spikes are reconstructed exactly on the
    host: s_t = [th_t - b*th_{t-1} > 0.5] (spike contribution is 1.0,
    bf16 rounding noise << 0.5).
  - Half-tile pipelining overlaps POOL(h1) with DVE(h0); 3-deep X prefetch
    absorbs in-DMA jitter.

x is pre-transposed on the host to [T, E] per core so each time step is one
contiguous 1 MiB DMA row. The kernel is bit-exact vs the f32 reference
(0 mismatched spikes / 67M on the harness input).
"""

import sys

if "/opt/trn_rl_repo" not in sys.path:
    sys.path.insert(0, "/opt/trn_rl_repo")

import numpy as np

B, C, N, T = 8, 64, 4096, 32
E = C * N  # 262144 elements per core
P = 128
F = E // P  # 2048
H = F // 2
NCORES = 8

_REG = {}
_COMPILED = {}


def _register_ops():
    """Register the fused LIF custom-DVE ops at runtime (append-only)."""
    if _REG:
        return _REG
    from concourse import dve_ops
    from concourse.dve_spec import C0, Spec, Src0, Src1, lower, select
    from concourse.dve_uop import DveOpSpec

    existing = {op.name: op for op in dve_ops.OPS}

    def reg(name, spec):
        if name in existing:
            return existing[name]
        row = dve_ops._CUSTOM_DVE_ROW_BASE + len(dve_ops.OPS)
        assert row < 0x20, "custom DVE row field overflow"
        dve_ops._SUB_OPCODE_FOR_NAME[name] = row
        shas = {}
        for ver in ("v3", "v4"):
            s = DveOpSpec(
                name=name,
                opcode=row,
                uops=lower(spec, ver=ver),
                rd1_en=dve_ops.has_src1(spec),
            )
            shas[ver] = s.sha(ver)
        op = dve_ops.DveOp(name, spec, False, uops_sha=shas)
        dve_ops.OPS.append(op)
        dve_ops.CUSTOM_DVE_SPECS[name] = spec
        return op

    # out = a * (in0 - in1 if in0 >= in1 else in0)   (in0=MP, in1=TH, s0=a)
    lif_mem = reg(
        "LIF_MEM_ANT",
        Spec(
            body=select(Src0 >= Src1, Src0 - Src1, Src0) * C0,
            reference=lambda in0, in1, s0, s1, imm2: (
                np.where(in0 >= in1, in0 - in1, in0) * np.float32(s0)
            ).astype(np.float32),
        ),
    )
    # out = b * in0 + (in1 >= in0)                   (in0=TH, in1=MP, s0=b)
    lif_thr = reg(
        "LIF_THR_ANT",
        Spec(
            body=Src0 * C0 + (Src1 >= Src0),
            reference=lambda in0, in1, s0, s1, imm2: (
                in0 * np.float32(s0) + (in1 >= in0).astype(np.float32)
            ).astype(np.float32),
        ),
    )
    # out = b * in1 + (in0 >= in1)                   (in0=MP, in1=TH, s0=b)
    # operand-swapped variant of LIF_THR so TH can ride the PSUM port
    lif_thr2 = reg(
        "LIF_THR2_ANT",
        Spec(
            body=Src1 * C0 + (Src0 >= Src1),
            reference=lambda in0, in1, s0, s1, imm2: (
                in1 * np.float32(s0) + (in0 >= in1).astype(np.float32)
            ).astype(np.float32),
        ),
    )
    _REG["mem"] = lif_mem
    _REG["thr"] = lif_thr
    _REG["thr2"] = lif_thr2
    return _REG


def _build(a, b):
    """Build the SPMD Bass graph. a/b are python floats holding exact f32
    decay values; they are baked into the custom-op scalar immediates."""
    import concourse.bass as bass
    import concourse.mybir as mybir

    ops = _register_ops()

    nc = bass.Bass(trn_type="TRN2")
    x = nc.declare_dram_parameter("x", [T, P, F], mybir.dt.float32, isOutput=False)
    out = nc.declare_dram_parameter("out", [T, P, F], mybir.dt.bfloat16, isOutput=True)

    from contextlib import ExitStack

    es = ExitStack()
    sb = lambda n, dt: es.enter_context(nc.sbuf_tensor(n, [P, F], dt))
    xb = [
        sb("x0", mybir.dt.float32),
        sb("x1", mybir.dt.float32),
        sb("x2", mybir.dt.float32),
    ]
    g = sb("g", mybir.dt.float32)
    mp = sb("mp", mybir.dt.float32)
    th = [
        es.enter_context(nc.psum_tensor("th0", [P, F], mybir.dt.float32)),
        es.enter_context(nc.psum_tensor("th1", [P, F], mybir.dt.float32)),
    ]
    tho = [sb("tho0", mybir.dt.bfloat16), sb("tho1", mybir.dt.bfloat16)]
    din = es.enter_context(nc.semaphore("din"))
    dinb = es.enter_context(nc.semaphore("dinb"))
    dout = es.enter_context(nc.semaphore("dout"))
    vsem = es.enter_context(nc.semaphore("vsem"))
    asem = es.enter_context(nc.semaphore("asem"))
    psem = es.enter_context(nc.semaphore("psem"))

    def half(buf, h):
        return buf[:, h * H : (h + 1) * H]

    with es, nc.Block() as block:

        @block.sync
        def _(sync):
            # t=0 load is split across the two HWDGE rings (h0 here, h1 on
            # the scalar ring) so the first DVE pair starts sooner
            sync.dma_start(half(xb[0], 0), x[0, :, 0:H]).then_inc(din, 16)
            for t in range(1, T):
                if t == 3:
                    # xb[0] was read by the t=0 DVE pairs (not POOL)
                    sync.wait_ge(vsem, 2)
                elif t >= 4:
                    # X buffer t%3 last read by POOL adds of step t-3
                    sync.wait_ge(psem, 2 * (t - 4) + 2)
                sync.dma_start(xb[t % 3][:, :], x[t, :, :]).then_inc(din, 16)

        @block.gpsimd
        def _(gpsimd):
            # step 0 has no ADD: the DVE pairs read X_0 directly (G = 0)
            for t in range(1, T):
                for h in range(2):
                    # G[h]/MP[h] free after the (h,t-1) DVE pair
                    gpsimd.wait_ge(vsem, 2 * (t - 1) + h + 1)
                    if h == 0:
                        gpsimd.wait_ge(din, 16 * (t + 1))
                    gpsimd.tensor_tensor(
                        half(mp, h),
                        half(g, h),
                        half(xb[t % 3], h),
                        mybir.AluOpType.add,
                    ).then_inc(psem, 1)

        @block.vector
        def _(vector):
            # split theta memset so the h0 pair isn't gated on the h1 half;
            # G needs no init (first write is MEM at t=0)
            nc.vector.memset(half(th[0], 0), 1.0)
            nc.vector.memset(half(th[0], 1), 1.0)
            for t in range(T):
                for h in range(2):
                    if t == 0:
                        # pairs read X_0 directly (membrane starts at 0)
                        vector.wait_ge(din if h == 0 else dinb, 16)
                    else:
                        vector.wait_ge(psem, 2 * (t - 1) + h + 1)
                    if t >= 2:
                        # Op3 writes TH[(t+1)%2][h]; ACT copy(h,t-2) read it
                        vector.wait_ge(asem, 2 * (t - 2) + h + 1)
                    # Op3: TH' = b*TH + (MP >= TH). At t = T-1 theta is
                    # pure output (no later step reads it), so write bf16
                    # straight to the staging buffer and skip the ACT cast.
                    thr_inst = nc.vector._custom_dve(
                        ops["thr2"],
                        out=half(tho[(T - 1) % 2], h)
                        if t == T - 1
                        else half(th[(t + 1) % 2], h),
                        in0=half(xb[0], h) if t == 0 else half(mp, h),
                        in1=half(th[t % 2], h),
                        s0=b,
                    )
                    if t == T - 1:
                        # G is dead after the last step: skip MEM
                        thr_inst.then_inc(vsem, 1)
                    else:
                        # Op2: G = a * select(MP >= TH, MP - TH, MP)
                        nc.vector._custom_dve(
                            ops["mem"],
                            out=half(g, h),
                            in0=half(xb[0], h) if t == 0 else half(mp, h),
                            in1=half(th[t % 2], h),
                            s0=a,
                        ).then_inc(vsem, 1)

        @block.scalar
        def _(scalar):
            # second half of the t=0 input load rides this ring
            scalar.dma_start(half(xb[0], 1), x[0, :, H:F]).then_inc(dinb, 16)
            for t in range(T):
                for h in range(2):
                    scalar.wait_ge(vsem, 2 * t + h + 1)
                    if t >= 2:
                        scalar.wait_ge(dout, 16 * (2 * (t - 2) + h + 1))
                    if t == T - 1:
                        # THR2 wrote bf16 into tho directly; vsem-gated DMA
                        # of DVE output is the original (proven) pattern
                        scalar.dma_start(
                            out[t, :, h * H : (h + 1) * H], half(tho[t % 2], h)
                        ).then_inc(dout, 16)
                        continue
                    # downcast f32 -> bf16 on ScalarE, then DMA out. The HWDGE
                    # descriptor post is a sequencer op that runs ahead of the
                    # ACT datapath, so self-wait on the copy's completion
                    # event before issuing the DMA.
                    nc.scalar.copy(
                        half(tho[t % 2], h), half(th[(t + 1) % 2], h)
                    ).then_inc(asem, 1)
                    scalar.wait_ge(asem, 2 * t + h + 1)
                    scalar.dma_start(
                        out[t, :, h * H : (h + 1) * H], half(tho[t % 2], h)
                    ).then_inc(dout, 16)

    # populate .instr bytes for InstISA subclasses (InstCustomDveAnt) —
    # raw Bass skips this pass; without it walrus fails "ISA wrong length"
    mybir.codegen_inst_isa_subclasses(nc)
    return nc


def _get_compiled(a, b):
    key = (a, b)
    if key not in _COMPILED:
        _COMPILED[key] = _build(a, b)
    return _COMPILED[key]


def _decays(tau_mem, tau_thresh):
    # match the reference bit-for-bit: f32 exp(-1/tau) computed via jax
    import jax.numpy as jnp

    a = float(jnp.exp(-1.0 / jnp.asarray(tau_mem, jnp.float32)))
    b = float(jnp.exp(-1.0 / jnp.asarray(tau_thresh, jnp.float32)))
    return a, b


def kernel(x, tau_mem, tau_thresh):
    from concourse.bass_utils import run_bass_kernel_spmd

    a, b = _decays(tau_mem, tau_thresh)
    x = np.asarray(x, dtype=np.float32)
    assert x.shape == (B, C, N, T), x.shape

    in_maps = []
    for i in range(NCORES):
        # [C,N,T] -> [E,T] -> [T,E] contiguous -> [T,P,F]
        xi = np.ascontiguousarray(x[i].reshape(E, T).T).reshape(T, P, F)
        in_maps.append({"x": xi})

    res = run_bass_kernel_spmd(_get_compiled(a, b), in_maps, list(range(NCORES)))

    out = np.empty((B, C, N, T), dtype=np.float32)
    for i in range(NCORES):
        th = np.asarray(res.results[i]["out"]).astype(np.float64).reshape(T, E)
        thprev = np.empty_like(th)
        thprev[0] = 1.0
        thprev[1:] = th[:-1]
        s = (th - b * thprev) > 0.5  # [T, E]
        out[i] = s.T.astype(np.float32).reshape(C, N, T)
    return out


# revision 19
# speedup vs baseline: 1.2108x; 1.0036x over previous
"""AdaptiveLIFNeuron Trainium2 kernel (8 NeuronCores, data-parallel over batch).

Reference recurrence per element (b, c, n), t = 0..T-1:
    m  = m * a + x_t          (a = exp(-1/tau_mem))
    s  = [m >= th]            (heaviside)
    m  = m - th * s           (reset by subtraction)
    th = th * b + s           (b = exp(-1/tau_thresh))
    out_t = s

Device strategy (per core, one batch element, E = C*N = 262144 elements laid
out as [128 partitions x 2048 free]):
  - GPSIMD (POOL) computes MP = G + X_t with a stock tensor_tensor add,
    running concurrently with the DVE (verified: DVE custom ops whose second
    operand is in PSUM leave the shared second SBUF port free).
  - DVE runs two runtime-registered custom ops per step:
      LIF_THR2: TH' = b*TH + (MP >= TH)       (TH state lives in PSUM)
      LIF_MEM:  G   = a*select(MP>=TH, MP-TH, MP)
  - ScalarE casts TH' (PSUM f32) -> bf16 SBUF, then its HWDGE ring DMAs the
    bf16 threshold sequence out. Spikes are reconstructed exactly on the
    host: s_t = [th_t - b*th_{t-1} > 0.5] (spike contribution is 1.0,
    bf16 rounding noise << 0.5).
  - Half-tile pipelining overlaps POOL(h1) with DVE(h0); 3-deep X prefetch
    absorbs in-DMA jitter.

x is pre-transposed on the host to [T, E] per core so each time step is one
contiguous 1 MiB DMA row. The kernel is bit-exact vs the f32 reference
(0 mismatched spikes / 67M on the harness input).
"""

import sys

if "/opt/trn_rl_repo" not in sys.path:
    sys.path.insert(0, "/opt/trn_rl_repo")

import numpy as np

B, C, N, T = 8, 64, 4096, 32
E = C * N  # 262144 elements per core
P = 128
F = E // P  # 2048
H = F // 2
NCORES = 8

_REG = {}
_COMPILED = {}


def _register_ops():
    """Register the fused LIF custom-DVE ops at runtime (append-only)."""
    if _REG:
        return _REG
    from concourse import dve_ops
    from concourse.dve_spec import C0, Spec, Src0, Src1, lower, select
    from concourse.dve_uop import DveOpSpec

    existing = {op.name: op for op in dve_ops.OPS}

    def reg(name, spec):
        if name in existing:
            return existing[name]
        row = dve_ops._CUSTOM_DVE_ROW_BASE + len(dve_ops.OPS)
        assert row < 0x20, "custom DVE row field overflow"
        dve_ops._SUB_OPCODE_FOR_NAME[name] = row
        shas = {}
        for ver in ("v3", "v4"):
            s = DveOpSpec(
                name=name,
                opcode=row,
                uops=lower(spec, ver=ver),
                rd1_en=dve_ops.has_src1(spec),
            )
            shas[ver] = s.sha(ver)
        op = dve_ops.DveOp(name, spec, False, uops_sha=shas)
        dve_ops.OPS.append(op)
        dve_ops.CUSTOM_DVE_SPECS[name] = spec
        return op

    # out = a * (in0 - in1 if in0 >= in1 else in0)   (in0=MP, in1=TH, s0=a)
    lif_mem = reg(
        "LIF_MEM_ANT",
        Spec(
            body=select(Src0 >= Src1, Src0 - Src1, Src0) * C0,
            reference=lambda in0, in1, s0, s1, imm2: (
                np.where(in0 >= in1, in0 - in1, in0) * np.float32(s0)
            ).astype(np.float32),
        ),
    )
    # out = b * in0 + (in1 >= in0)                   (in0=TH, in1=MP, s0=b)
    lif_thr = reg(
        "LIF_THR_ANT",
        Spec(
            body=Src0 * C0 + (Src1 >= Src0),
            reference=lambda in0, in1, s0, s1, imm2: (
                in0 * np.float32(s0) + (in1 >= in0).astype(np.float32)
            ).astype(np.float32),
        ),
    )
    # out = b * in1 + (in0 >= in1)                   (in0=MP, in1=TH, s0=b)
    # operand-swapped variant of LIF_THR so TH can ride the PSUM port
    lif_thr2 = reg(
        "LIF_THR2_ANT",
        Spec(
            body=Src1 * C0 + (Src0 >= Src1),
            reference=lambda in0, in1, s0, s1, imm2: (
                in1 * np.float32(s0) + (in0 >= in1).astype(np.float32)
            ).astype(np.float32),
        ),
    )
    from concourse.dve_spec import One

    # t=0 variants with theta = 1.0 baked in (single tensor stream: X only)
    # out = b + (in0 >= 1)
    lif_thr_t0 = reg(
        "LIF_THR_T0_ANT",
        Spec(
            body=(Src0 >= One) + C0,
            reference=lambda in0, in1, s0, s1, imm2: (
                (in0 >= 1.0).astype(np.float32) + np.float32(s0)
            ).astype(np.float32),
        ),
    )
    # out = a * (in0 - 1 if in0 >= 1 else in0)
    lif_mem_t0 = reg(
        "LIF_MEM_T0_ANT",
        Spec(
            body=select(Src0 >= One, Src0 - One, Src0) * C0,
            reference=lambda in0, in1, s0, s1, imm2: (
                np.where(in0 >= 1.0, in0 - np.float32(1.0), in0) * np.float32(s0)
            ).astype(np.float32),
        ),
    )
    _REG["mem"] = lif_mem
    _REG["thr"] = lif_thr
    _REG["thr2"] = lif_thr2
    _REG["thr_t0"] = lif_thr_t0
    _REG["mem_t0"] = lif_mem_t0
    return _REG


def _build(a, b):
    """Build the SPMD Bass graph. a/b are python floats holding exact f32
    decay values; they are baked into the custom-op scalar immediates."""
    import concourse.bass as bass
    import concourse.mybir as mybir

    ops = _register_ops()

    nc = bass.Bass(trn_type="TRN2")
    x = nc.declare_dram_parameter("x", [T, P, F], mybir.dt.float32, isOutput=False)
    out = nc.declare_dram_parameter("out", [T, P, F], mybir.dt.bfloat16, isOutput=True)

    from contextlib import ExitStack

    es = ExitStack()
    sb = lambda n, dt: es.enter_context(nc.sbuf_tensor(n, [P, F], dt))
    xb = [
        sb("x0", mybir.dt.float32),
        sb("x1", mybir.dt.float32),
        sb("x2", mybir.dt.float32),
    ]
    g = sb("g", mybir.dt.float32)
    mp = sb("mp", mybir.dt.float32)
    th = [
        es.enter_context(nc.psum_tensor("th0", [P, F], mybir.dt.float32)),
        es.enter_context(nc.psum_tensor("th1", [P, F], mybir.dt.float32)),
    ]
    tho = [sb("tho0", mybir.dt.bfloat16), sb("tho1", mybir.dt.bfloat16)]
    din = es.enter_context(nc.semaphore("din"))
    dinb = es.enter_context(nc.semaphore("dinb"))
    dout = es.enter_context(nc.semaphore("dout"))
    vsem = es.enter_context(nc.semaphore("vsem"))
    asem = es.enter_context(nc.semaphore("asem"))
    psem = es.enter_context(nc.semaphore("psem"))

    def half(buf, h):
        return buf[:, h * H : (h + 1) * H]

    with es, nc.Block() as block:

        @block.sync
        def _(sync):
            # t=0 load is split across the two HWDGE rings (h0 here, h1 on
            # the scalar ring) so the first DVE pair starts sooner
            sync.dma_start(half(xb[0], 0), x[0, :, 0:H]).then_inc(din, 16)
            for t in range(1, T):
                if t == 3:
                    # xb[0] was read by the t=0 DVE pairs (not POOL)
                    sync.wait_ge(vsem, 2)
                elif t >= 4:
                    # X buffer t%3 last read by POOL adds of step t-3
                    sync.wait_ge(psem, 2 * (t - 4) + 2)
                sync.dma_start(xb[t % 3][:, :], x[t, :, :]).then_inc(din, 16)

        @block.gpsimd
        def _(gpsimd):
            # step 0 has no ADD: the DVE pairs read X_0 directly (G = 0)
            for t in range(1, T):
                for h in range(2):
                    # G[h]/MP[h] free after the (h,t-1) DVE pair
                    gpsimd.wait_ge(vsem, 2 * (t - 1) + h + 1)
                    if h == 0:
                        gpsimd.wait_ge(din, 16 * (t + 1))
                    gpsimd.tensor_tensor(
                        half(mp, h),
                        half(g, h),
                        half(xb[t % 3], h),
                        mybir.AluOpType.add,
                    ).then_inc(psem, 1)

        @block.vector
        def _(vector):
            # no state init needed at all: t=0 ops bake theta=1 in as a
            # constant (so th[] is first written, never read, at t=0) and
            # G's first write is the t=0 MEM
            for t in range(T):
                for h in range(2):
                    if t == 0:
                        # pairs read X_0 directly (membrane starts at 0)
                        vector.wait_ge(din if h == 0 else dinb, 16)
                    else:
                        vector.wait_ge(psem, 2 * (t - 1) + h + 1)
                    if t >= 2:
                        # Op3 writes TH[(t+1)%2][h]; ACT copy(h,t-2) read it
                        vector.wait_ge(asem, 2 * (t - 2) + h + 1)
                    # Op3: TH' = b*TH + (MP >= TH). At t = T-1 theta is
                    # pure output (no later step reads it), so write bf16
                    # straight to the staging buffer and skip the ACT cast.
                    # At t = 0 theta is the constant 1.0 (single-stream op).
                    if t == 0:
                        thr_inst = nc.vector._custom_dve(
                            ops["thr_t0"],
                            out=half(th[1], h),
                            in0=half(xb[0], h),
                            s0=b,
                        )
                    else:
                        thr_inst = nc.vector._custom_dve(
                            ops["thr2"],
                            out=half(tho[(T - 1) % 2], h)
                            if t == T - 1
                            else half(th[(t + 1) % 2], h),
                            in0=half(mp, h),
                            in1=half(th[t % 2], h),
                            s0=b,
                        )
                    if t == T - 1:
                        # G is dead after the last step: skip MEM
                        thr_inst.then_inc(vsem, 1)
                    else:
                        # Op2: G = a * select(MP >= TH, MP - TH, MP)
                        if t == 0:
                            nc.vector._custom_dve(
                                ops["mem_t0"],
                                out=half(g, h),
                                in0=half(xb[0], h),
                                s0=a,
                            ).then_inc(vsem, 1)
                        else:
                            nc.vector._custom_dve(
                                ops["mem"],
                                out=half(g, h),
                                in0=half(mp, h),
                                in1=half(th[t % 2], h),
                                s0=a,
                            ).then_inc(vsem, 1)

        @block.scalar
        def _(scalar):
            # second half of the t=0 input load rides this ring
            scalar.dma_start(half(xb[0], 1), x[0, :, H:F]).then_inc(dinb, 16)
            for t in range(T):
                for h in range(2):
                    scalar.wait_ge(vsem, 2 * t + h + 1)
                    if t >= 2:
                        scalar.wait_ge(dout, 16 * (2 * (t - 2) + h + 1))
                    if t == T - 1:
                        # THR2 wrote bf16 into tho directly; vsem-gated DMA
                        # of DVE output is the original (proven) pattern
                        scalar.dma_start(
                            out[t, :, h * H : (h + 1) * H], half(tho[t % 2], h)
                        ).then_inc(dout, 16)
                        continue
                    # downcast f32 -> bf16 on ScalarE, then DMA out. The HWDGE
                    # descriptor post is a sequencer op that runs ahead of the
                    # ACT datapath, so self-wait on the copy's completion
                    # event before issuing the DMA.
                    nc.scalar.copy(
                        half(tho[t % 2], h), half(th[(t + 1) % 2], h)
                    ).then_inc(asem, 1)
                    scalar.wait_ge(asem, 2 * t + h + 1)
                    scalar.dma_start(
                        out[t, :, h * H : (h + 1) * H], half(tho[t % 2], h)
                    ).then_inc(dout, 16)

    # populate .instr bytes for InstISA subclasses (InstCustomDveAnt) —
    # raw Bass skips this pass; without it walrus fails "ISA wrong length"
    mybir.codegen_inst_isa_subclasses(nc)
    return nc


def _get_compiled(a, b):
    key = (a, b)
    if key not in _COMPILED:
        _COMPILED[key] = _build(a, b)
    return _COMPILED[key]


def _decays(tau_mem, tau_thresh):
    # match the reference bit-for-bit: f32 exp(-1/tau) computed via jax
    import jax.numpy as jnp

    a = float(jnp.exp(-1.0 / jnp.asarray(tau_mem, jnp.float32)))
    b = float(jnp.exp(-1.0 / jnp.asarray(tau_thresh, jnp.float32)))
    return a, b


def kernel(x, tau_mem, tau_thresh):
    from concourse.bass_utils import run_bass_kernel_spmd

    a, b = _decays(tau_mem, tau_thresh)
    x = np.asarray(x, dtype=np.float32)
    assert x.shape == (B, C, N, T), x.shape

    in_maps = []
    for i in range(NCORES):
        # [C,N,T] -> [E,T] -> [T,E] contiguous -> [T,P,F]
        xi = np.ascontiguousarray(x[i].reshape(E, T).T).reshape(T, P, F)
        in_maps.append({"x": xi})

    res = run_bass_kernel_spmd(_get_compiled(a, b), in_maps, list(range(NCORES)))

    out = np.empty((B, C, N, T), dtype=np.float32)
    for i in range(NCORES):
        th = np.asarray(res.results[i]["out"]).astype(np.float64).reshape(T, E)
        thprev = np.empty_like(th)
        thprev[0] = 1.0
        thprev[1:] = th[:-1]
        s = (th - b * thprev) > 0.5  # [T, E]
        out[i] = s.T.astype(np.float32).reshape(C, N, T)
    return out
